# revision 16
# baseline (speedup 1.0000x reference)
import sys
if '/opt/trn_rl_repo' not in sys.path:
    sys.path.insert(0, '/opt/trn_rl_repo')
import numpy as np
import concourse.bacc as bacc
import concourse.mybir as mybir
import concourse.tile as tile
from concourse.bass_utils import run_bass_kernel_spmd

dt = mybir.dt
AF = mybir.ActivationFunctionType
OP = mybir.AluOpType
BF16 = dt.np(dt.bfloat16)

# problem shapes (hardcoded per spec)
T = 2048
D = 2048
H = 16
HD = 128
FFN = 8192
CORES = 8
HPC = H // CORES          # heads per core = 2
E = HPC * HD              # per-core attention feature width = 256
DSH = D // CORES          # d-shard rows per core = 256
FSH = FFN // CORES        # ffn rows per core = 1024
CH = 512                  # t-chunk
EPS = float(np.finfo(np.float32).eps)

_CACHE = {}
LAST_RESULT = None


def _attn_band(docs):
    """Per t-chunk list of (st, full, mask[128,CH] or None)."""
    docs = np.asarray(docs).astype(np.int64)
    is_sorted = bool(np.all(np.diff(docs) >= 0))
    out = []
    for tc in range(T // CH):
        t0, t1 = tc * CH, (tc + 1) * CH
        if is_sorted:
            s_lo = int(np.searchsorted(docs, docs[t0], side='left'))
            st_lo = s_lo // 128
        else:
            st_lo = 0
        st_hi = (t1 - 1) // 128
        tiles = []
        for st in range(st_lo, st_hi + 1):
            s0, s1 = st * 128, (st + 1) * 128
            m = (np.arange(t0, t1)[None, :] >= np.arange(s0, s1)[:, None]) \
                & (docs[None, t0:t1] == docs[s0:s1, None])
            if not m.any():
                continue
            full = bool(m.all())
            tiles.append((st, full, None if full else m.astype(np.float32)))
        out.append(tiles)
    return out


def _build(band, scale, key_offset):
    """Build the SPMD Bass program (identical for all cores)."""
    nc = bacc.Bacc("TRN2", target_bir_lowering=False, debug=False,
                   num_devices=CORES)
    f32, f32r, bf = dt.float32, dt.float32r, dt.bfloat16

    mask_idx = {}
    for tcc, tiles in enumerate(band):
        for (st, full, m) in tiles:
            if not full:
                mask_idx[(tcc, st)] = len(mask_idx)
    n_masks = max(len(mask_idx), 1)

    # ---- DRAM I/O ----
    xT_d   = nc.dram_tensor("xT",   [D, T], bf, kind="ExternalInput")
    xTs_d  = nc.dram_tensor("xTs",  [DSH, T], f32, kind="ExternalInput")
    wqkv_d = nc.dram_tensor("wqkvT", [D, 3 * E], bf, kind="ExternalInput")
    wo_d   = nc.dram_tensor("woT",  [D, DSH], bf, kind="ExternalInput")
    wfc_d  = nc.dram_tensor("wfcT", [D, FSH], bf, kind="ExternalInput")
    wpr_d  = nc.dram_tensor("wpr",  [FSH, D], bf, kind="ExternalInput")
    wg_d   = nc.dram_tensor("wgT",  [D, 128], bf, kind="ExternalInput")
    ve_d   = nc.dram_tensor("veS",  [T, E], bf, kind="ExternalInput")
    ropeA_d = nc.dram_tensor("ropeA", [HD, T], bf, kind="ExternalInput")
    ropeB_d = nc.dram_tensor("ropeB", [HD, T], bf, kind="ExternalInput")
    idb_d  = nc.dram_tensor("identb", [128, 128], bf, kind="ExternalInput")
    ones_d = nc.dram_tensor("ones", [128, 1], f32r, kind="ExternalInput")
    onesb_d = nc.dram_tensor("onesb", [128, 1], bf, kind="ExternalInput")
    eps_d  = nc.dram_tensor("epsb", [128, 1], f32, kind="ExternalInput")
    mask_d = nc.dram_tensor("masks", [n_masks, 128, CH], bf, kind="ExternalInput")

    mlp_d  = nc.dram_tensor("mlp_partialT", [D, T], f32, kind="ExternalOutput")
    x1s_d  = nc.dram_tensor("x1T_shard", [DSH, T], f32, kind="ExternalOutput")

    NT = T // CH
    ND = D // 128
    NE = DSH // 128
    NF = FSH // 128
    q_of, k_of, v_of = 0, HPC, 2 * HPC

    with tile.TileContext(nc) as tc_:
        with tc_.tile_pool(name="const", bufs=1) as const, \
             tc_.tile_pool(name="dram", bufs=1, space="DRAM") as dram, \
             tc_.tile_pool(name="sDW", bufs=1) as sDW:
            identb = const.tile([128, 128], bf)
            nc.sync.dma_start(identb[:], idb_d.ap())
            ones = const.tile([128, 1], f32r)
            nc.sync.dma_start(ones[:], ones_d.ap())
            onesb = const.tile([128, 1], bf)
            nc.sync.dma_start(onesb[:], onesb_d.ap())
            epsb = const.tile([128, 1], f32)
            nc.sync.dma_start(epsb[:], eps_d.ap())

            # MLP weights prefetched at kernel start (overlaps stages A-C)
            wfc = sDW.tile([128, ND, FSH], bf)
            nc.sync.dma_start(
                wfc[:], wfc_d.ap().rearrange("(n p) f -> p n f", p=128))
            wpr = sDW.tile([128, NF, D], bf)
            nc.sync.dma_start(
                wpr[:], wpr_d.ap().rearrange("(n p) d -> p n d", p=128))

            cc_y_in = [dram.tile([E, CH], bf, name=f"ccyi{t}")
                       for t in range(NT)]
            cc_y_out = [dram.tile([CORES * E, CH], bf, addr_space="Shared",
                                  name=f"ccyo{t}") for t in range(NT)]
            cc_n_in = dram.tile([1, T], f32)
            cc_n_out = dram.tile([1, T], f32, addr_space="Shared")
            cc_x_in = [dram.tile([DSH, CH], bf, name=f"ccxi{t}")
                       for t in range(NT)]
            cc_x_out = [dram.tile([CORES * DSH, CH], bf, addr_space="Shared",
                                  name=f"ccxo{t}") for t in range(NT)]

            with tc_.tile_pool(name="pBC", bufs=1) as pBC:
                x1T = pBC.tile([128, NE, T], f32)

                with tc_.tile_pool(name="pAB", bufs=1) as pAB:
                    qkvT = pAB.tile([128, 3 * HPC, T], bf)
                    # gate rows at partitions 0,32 (ve) and 64,96 (attn)
                    gates = pAB.tile([128, T], bf)

                    # ===== Stage A: qkv + gates on raw x; rms-norm commutes
                    # with the projection, folded into psum eviction =====
                    with tc_.tile_pool(name="stA", bufs=1) as sA, \
                         tc_.tile_pool(name="stA2", bufs=2) as sA2, \
                         tc_.tile_pool(name="psA", bufs=4, space="PSUM") as psA, \
                         tc_.tile_pool(name="psRow", bufs=2, space="PSUM") as psRow:
                        wq = sA.tile([128, ND, 3 * E], bf)
                        nc.sync.dma_start(
                            wq[:], wqkv_d.ap().rearrange("(n p) e -> p n e", p=128))
                        wg = sA.tile([128, ND, 128], bf)
                        nc.sync.dma_start(
                            wg[:], wg_d.ap().rearrange("(n p) e -> p n e", p=128))
                        xTr = xT_d.ap().rearrange("(n p) t -> p n t", p=128)
                        for t in range(NT):
                            ts = slice(t * CH, (t + 1) * CH)
                            xt = sA2.tile([128, ND, CH], bf, tag="xt")
                            nc.sync.dma_start(xt[:], xTr[:, :, ts])
                            pr = psRow.tile([1, CH], f32, tag="row")
                            for n in range(ND):
                                sq = sA2.tile([128, CH], f32r, tag="sq")
                                nc.scalar.activation(sq[:], xt[:, n], AF.Square)
                                nc.tensor.matmul(pr[:], ones[:], sq[:],
                                                 start=(n == 0), stop=(n == ND - 1))
                            sd = sA2.tile([1, CH], f32, tag="sd")
                            nc.scalar.activation(sd[:], pr[:], AF.Sqrt,
                                                 bias=epsb[0:1, :], scale=1.0 / D)
                            rcp = sA2.tile([1, CH], f32, tag="rcp")
                            nc.vector.reciprocal_approx_fast(rcp[:], sd[:])
                            inv = sA2.tile([1, CH], f32r, tag="inv")
                            nc.vector.tensor_copy(inv[:], rcp[:])
                            rep = sA2.tile([128, CH], f32r, tag="rep")
                            nc.gpsimd.partition_broadcast(rep[:], inv[:])
                            for m in range(3 * HPC):
                                pq = psA.tile([128, CH], f32, tag="mm")
                                for n in range(ND):
                                    nc.tensor.matmul(
                                        pq[:], wq[:, n, m * 128:(m + 1) * 128],
                                        xt[:, n], start=(n == 0), stop=(n == ND - 1))
                                nc.vector.tensor_tensor(qkvT[:, m, ts], pq[:],
                                                        rep[:], OP.mult)
                            pg = psA.tile([128, CH], f32, tag="g", bufs=2)
                            for n in range(ND):
                                nc.tensor.matmul(pg[:], wg[:, n], xt[:, n],
                                                 start=(n == 0), stop=(n == ND - 1))
                            gm = sA2.tile([128, CH], f32, tag="gm")
                            nc.vector.tensor_tensor(gm[:], pg[:], rep[:], OP.mult)
                            nc.scalar.activation(gates[:, ts], gm[:], AF.Sigmoid)

                    # ===== Stage B: attention; y AllGather chunked by t =====
                    with tc_.tile_pool(name="stB", bufs=1) as sB, \
                         tc_.tile_pool(name="stB2", bufs=2) as sB2, \
                         tc_.tile_pool(name="psB", bufs=2, space="PSUM") as psB, \
                         tc_.tile_pool(name="psBa", bufs=2, space="PSUM") as psBa, \
                         tc_.tile_pool(name="psBr", bufs=2, space="PSUM") as psBr:
                        ropeA = sB.tile([HD, T], bf)
                        nc.sync.dma_start(ropeA[:], ropeA_d.ap())
                        ropeB = sB.tile([HD, T], bf)
                        nc.sync.dma_start(ropeB[:], ropeB_d.ap())
                        gcol = sB.tile([128, T // 128, 2], bf)
                        for st in range(T // 128):
                            pt = psB.tile([128, 128], bf, tag="vt")
                            nc.tensor.transpose(
                                pt[:], gates[:, st * 128:(st + 1) * 128], identb[:])
                            nc.vector.tensor_copy(gcol[:, st, 0:1], pt[:, 0:1])
                            nc.vector.tensor_copy(gcol[:, st, 1:2], pt[:, 32:33])

                        qrs, kfs, vuses = [], [], []
                        for h in range(HPC):
                            q = qkvT[:, q_of + h]
                            k = qkvT[:, k_of + h]
                            vt = qkvT[:, v_of + h]
                            hh = HD // 2
                            qr = sB.tile([128, T], bf, tag=f"qr{h}")
                            kr = sB.tile([128, T], bf, tag="kr")
                            tmp = sB.tile([128, T], bf, tag="tmp")
                            # rotary on raw q/k (rms-norm folded in after:
                            # rot(n*q) = n*rot(q)). ropeA = [cos;sin],
                            # ropeB = [sin;cos] for base-partition alignment.
                            for (src_, dst) in ((q, qr), (k, kr)):
                                x1, x2 = src_[0:hh, :], src_[hh:HD, :]
                                cosA, sinA = ropeA[0:hh, :], ropeA[hh:HD, :]
                                sinB, cosB = ropeB[0:hh, :], ropeB[hh:HD, :]
                                nc.gpsimd.tensor_tensor(tmp[0:hh, :], x2, sinA,
                                                        OP.mult)
                                nc.vector.tensor_tensor(dst[0:hh, :], x1, cosA,
                                                        OP.mult)
                                nc.vector.tensor_tensor(dst[0:hh, :], dst[0:hh, :],
                                                        tmp[0:hh, :], OP.add)
                                nc.gpsimd.tensor_tensor(tmp[hh:HD, :], x1, sinB,
                                                        OP.mult)
                                nc.vector.tensor_tensor(dst[hh:HD, :], x2, cosB,
                                                        OP.mult)
                                nc.vector.tensor_tensor(dst[hh:HD, :], dst[hh:HD, :],
                                                        tmp[hh:HD, :], OP.subtract)
                                # rms-norm factors (from raw src) applied to dst
                                for t in range(NT):
                                    ts = slice(t * CH, (t + 1) * CH)
                                    sq2 = sB2.tile([128, CH], f32r, tag="sq2")
                                    nc.scalar.activation(sq2[:], src_[:, ts],
                                                         AF.Square)
                                    pr2 = psBr.tile([1, CH], f32, tag="row")
                                    nc.tensor.matmul(pr2[:], ones[:], sq2[:],
                                                     start=True, stop=True)
                                    sd2 = sB2.tile([1, CH], f32, tag="sd2")
                                    nc.scalar.activation(sd2[:], pr2[:], AF.Sqrt,
                                                         bias=epsb[0:1, :],
                                                         scale=1.0 / HD)
                                    rcp2 = sB2.tile([1, CH], f32, tag="rcp2")
                                    nc.vector.reciprocal_approx_fast(rcp2[:], sd2[:])
                                    inv2 = sB2.tile([1, CH], f32r, tag="inv2")
                                    nc.vector.tensor_copy(inv2[:], rcp2[:])
                                    rep2 = sB2.tile([128, CH], f32r, tag="rep2")
                                    nc.gpsimd.partition_broadcast(rep2[:], inv2[:])
                                    nc.vector.tensor_tensor(dst[:, ts], dst[:, ts],
                                                            rep2[:], OP.mult)
                            # key_offset shift (copies on GpSimd, 1-input ops
                            # run at line rate there and DVE is the hot engine)
                            if key_offset:
                                kf = sB.tile([128, T], bf, tag=f"kf{h}")
                                a, b, c3 = HD // 4, HD // 2, 3 * HD // 4
                                nc.gpsimd.tensor_copy(kf[0:a, :], kr[0:a, :])
                                nc.gpsimd.tensor_copy(kf[b:c3, :], kr[b:c3, :])
                                nc.gpsimd.tensor_copy(kf[a:b, 1:T], kr[a:b, 0:T - 1])
                                nc.gpsimd.tensor_copy(kf[a:b, 0:1], kr[a:b, 0:1])
                                nc.gpsimd.tensor_copy(kf[c3:HD, 1:T],
                                                      kr[c3:HD, 0:T - 1])
                                nc.gpsimd.tensor_copy(kf[c3:HD, 0:1], kr[c3:HD, 0:1])
                            else:
                                kf = sB.tile([128, T], bf, tag=f"kf{h}")
                                nc.gpsimd.tensor_copy(kf[:], kr[:])
                            # v natural + ve gate
                            ve_nat = sB.tile([128, T // 128, HD], bf, tag="ve")
                            nc.sync.dma_start(
                                ve_nat[:],
                                ve_d.ap()[:, h * HD:(h + 1) * HD]
                                    .rearrange("(n p) e -> p n e", p=128))
                            vuse = sB.tile([128, T // 128, HD], bf, tag=f"vuse{h}")
                            for st in range(T // 128):
                                pv = psB.tile([128, HD], bf, tag="vt")
                                nc.tensor.transpose(
                                    pv[:], vt[:, st * 128:(st + 1) * 128], identb[:])
                                nc.vector.scalar_tensor_tensor(
                                    vuse[:, st], ve_nat[:, st],
                                    gcol[:, st, h:h + 1], pv[:], OP.mult, OP.add)
                            qrs.append(qr)
                            kfs.append(kf)
                            vuses.append(vuse)

                        # attention over the same-doc causal band, t outer so
                        # each finished chunk AllGathers immediately
                        yTt = None
                        for t in range(NT):
                            ts = slice(t * CH, (t + 1) * CH)
                            tiles = band[t]
                            yTt = sB.tile([128, HPC, CH], bf, tag="yTt", bufs=2)
                            for h in range(HPC):
                                qr, kf, vuse = qrs[h], kfs[h], vuses[h]
                                py = psBa.tile([128, CH], f32, tag="y")
                                pden = psBr.tile([1, CH], f32, tag="row")
                                for i, (st, full, _m) in enumerate(tiles):
                                    ps = psB.tile([128, CH], f32, tag="s")
                                    nc.tensor.matmul(
                                        ps[:], kf[:, st * 128:(st + 1) * 128],
                                        qr[:, ts], start=True, stop=True)
                                    pT = sB2.tile([128, CH], bf, tag="pT")
                                    nc.scalar.activation(pT[:], ps[:], AF.Exp,
                                                         scale=scale)
                                    if not full:
                                        mk = sB2.tile([128, CH], bf, tag="mk")
                                        nc.sync.dma_start(
                                            mk[:], mask_d.ap()[mask_idx[(t, st)]])
                                        nc.vector.tensor_tensor(pT[:], pT[:], mk[:],
                                                                OP.mult)
                                    nc.tensor.matmul(
                                        py[:], vuse[:, st], pT[:],
                                        start=(i == 0), stop=(i == len(tiles) - 1))
                                    nc.tensor.matmul(
                                        pden[:], onesb[:], pT[:],
                                        start=(i == 0), stop=(i == len(tiles) - 1))
                                rec = sB2.tile([1, CH], f32, tag="rec")
                                nc.vector.reciprocal_approx_fast(rec[:], pden[:])
                                grow = sB2.tile([1, CH], f32r, tag="grow")
                                nc.vector.tensor_copy(
                                    grow[:], gates[64 + 32 * h:65 + 32 * h, ts])
                                comb = sB2.tile([1, CH], f32r, tag="comb")
                                nc.vector.tensor_tensor(comb[:], rec[:], grow[:],
                                                        OP.mult)
                                repy = sB2.tile([128, CH], f32r, tag="repy")
                                nc.gpsimd.partition_broadcast(repy[:], comb[:])
                                nc.vector.tensor_tensor(yTt[:, h], py[:],
                                                        repy[:], OP.mult)
                            nc.sync.dma_start(
                                cc_y_in[t][:].rearrange("(h p) c -> p h c", p=128),
                                yTt[:])
                            nc.gpsimd.collective_compute(
                                "AllGather", OP.bypass,
                                replica_groups=[list(range(CORES))],
                                ins=[cc_y_in[t][:].opt()],
                                outs=[cc_y_out[t][:].opt()])

                # ===== Stage C: o-proj per chunk, norm, chunked AG(xn1) =====
                with tc_.tile_pool(name="stC", bufs=1) as sC, \
                     tc_.tile_pool(name="stC2", bufs=2) as sC2, \
                     tc_.tile_pool(name="psC", bufs=2, space="PSUM") as psC, \
                     tc_.tile_pool(name="psCr", bufs=2, space="PSUM") as psCr:
                    wo = sC.tile([128, ND, DSH], bf)
                    nc.sync.dma_start(
                        wo[:], wo_d.ap().rearrange("(n p) e -> p n e", p=128))
                    xs = sC.tile([128, NE, T], f32)
                    nc.sync.dma_start(
                        xs[:], xTs_d.ap().rearrange("(n p) t -> p n t", p=128))
                    nsum = sC.tile([1, T], f32)
                    for t in range(NT):
                        ts = slice(t * CH, (t + 1) * CH)
                        ygt = sC2.tile([128, ND, CH], bf, tag="ygt")
                        nc.sync.dma_start(
                            ygt[:],
                            cc_y_out[t][:].rearrange("(n p) c -> p n c", p=128))
                        for m in range(NE):
                            po = psC.tile([128, CH], f32, tag="o")
                            for n in range(ND):
                                nc.tensor.matmul(
                                    po[:], wo[:, n, m * 128:(m + 1) * 128],
                                    ygt[:, n], start=(n == 0), stop=(n == ND - 1))
                            nc.vector.tensor_tensor(x1T[:, m, ts], po[:],
                                                    xs[:, m, ts], OP.add)
                        pr3 = psCr.tile([1, CH], f32, tag="r3")
                        for m in range(NE):
                            sq3 = sC2.tile([128, CH], f32r, tag="sq3")
                            nc.scalar.activation(sq3[:], x1T[:, m, ts], AF.Square)
                            nc.tensor.matmul(pr3[:], ones[:], sq3[:],
                                             start=(m == 0), stop=(m == NE - 1))
                        nc.vector.tensor_copy(nsum[:, ts], pr3[:])
                    nc.sync.dma_start(
                        x1s_d.ap().rearrange("(n p) t -> p n t", p=128), x1T[:])
                    nc.sync.dma_start(cc_n_in[:], nsum[:])
                    nc.gpsimd.collective_compute(
                        "AllReduce", OP.add, replica_groups=[list(range(CORES))],
                        ins=[cc_n_in[:].opt()], outs=[cc_n_out[:].opt()])
                    nfull = sC.tile([1, T], f32)
                    nc.sync.dma_start(nfull[:], cc_n_out[:])
                    sd3 = sC.tile([1, T], f32)
                    nc.scalar.activation(sd3[:], nfull[:], AF.Sqrt,
                                         bias=epsb[0:1, :], scale=1.0 / D)
                    rcp3 = sC.tile([1, T], f32)
                    nc.vector.reciprocal_approx_fast(rcp3[:], sd3[:])
                    inv3 = sC.tile([1, T], f32r)
                    nc.vector.tensor_copy(inv3[:], rcp3[:])
                    rep3 = sC.tile([128, T], f32r)
                    nc.gpsimd.partition_broadcast(rep3[:], inv3[:])
                    xn1 = sC.tile([128, NE, T], bf)
                    for m in range(NE):
                        nc.vector.tensor_tensor(xn1[:, m], x1T[:, m], rep3[:],
                                                OP.mult)
                    for t in range(NT):
                        ts = slice(t * CH, (t + 1) * CH)
                        nc.sync.dma_start(
                            cc_x_in[t][:].rearrange("(n p) c -> p n c", p=128),
                            xn1[:, :, ts])
                        nc.gpsimd.collective_compute(
                            "AllGather", OP.bypass,
                            replica_groups=[list(range(CORES))],
                            ins=[cc_x_in[t][:].opt()], outs=[cc_x_out[t][:].opt()])

            # ===== Stage D: MLP (weights prefetched in sDW) =====
            with tc_.tile_pool(name="stD2", bufs=2) as sD2, \
                 tc_.tile_pool(name="psD", bufs=3, space="PSUM") as psD:
                mlpr = mlp_d.ap().rearrange("(n p) t -> p n t", p=128)
                for t in range(NT):
                    ts = slice(t * CH, (t + 1) * CH)
                    xnt = sD2.tile([128, ND, CH], bf, tag="xnt")
                    nc.sync.dma_start(
                        xnt[:],
                        cc_x_out[t][:].rearrange("(n p) c -> p n c", p=128))
                    hT = sD2.tile([128, NF, CH], bf, tag="hT")
                    for mf in range(NF):
                        ph = psD.tile([128, CH], f32, tag="h")
                        for n in range(ND):
                            nc.tensor.matmul(
                                ph[:], wfc[:, n, mf * 128:(mf + 1) * 128],
                                xnt[:, n], start=(n == 0), stop=(n == ND - 1))
                        hr = sD2.tile([128, CH], f32, tag="hr")
                        nc.scalar.activation(hr[:], ph[:], AF.Relu)
                        nc.vector.tensor_tensor(hT[:, mf], hr[:], hr[:], OP.mult)
                    for md in range(ND):
                        pm = psD.tile([128, CH], f32, tag="m")
                        for mf in range(NF):
                            nc.tensor.matmul(
                                pm[:], wpr[:, mf, md * 128:(md + 1) * 128],
                                hT[:, mf], start=(mf == 0), stop=(mf == NF - 1))
                        ot = sD2.tile([128, CH], f32, tag="ot")
                        nc.vector.tensor_copy(ot[:], pm[:])
                        nc.sync.dma_start(mlpr[:, md, ts], ot[:])

    nc.compile()
    return nc


def kernel(x, ve, qkvo_w, attn_gate_w, ve_gate_w, c_fc, c_proj,
           sa_lambdas, cos, sin, attn_scale, docs, key_offset):
    global LAST_RESULT
    x = np.asarray(x, np.float32)
    ve = np.asarray(ve, np.float32)
    qkvo_w = np.asarray(qkvo_w, np.float32)
    attn_gate_w = np.asarray(attn_gate_w, np.float32)
    ve_gate_w = np.asarray(ve_gate_w, np.float32)
    c_fc = np.asarray(c_fc, np.float32)
    c_proj = np.asarray(c_proj, np.float32)
    sa = np.asarray(sa_lambdas, np.float32)
    docs = np.asarray(docs)
    ko = int(np.asarray(key_offset))
    scale = float(np.asarray(attn_scale).reshape(-1)[0])

    band = _attn_band(docs)
    key = (docs.tobytes(), scale, ko)
    if key not in _CACHE:
        _CACHE[key] = _build(band, scale, ko)
    nc = _CACHE[key]

    xT = np.ascontiguousarray(x[0].T)                       # [D, T] f32
    w_qkv = sa[0] * qkvo_w[:3 * D]                          # [3D, D]
    w_o = sa[1] * qkvo_w[3 * D:]                            # [D, D]
    cosT = np.asarray(cos, np.float32).T
    sinT = np.asarray(sin, np.float32).T
    ropeA = np.ascontiguousarray(np.concatenate([cosT, sinT], 0)).astype(BF16)
    ropeB = np.ascontiguousarray(np.concatenate([sinT, cosT], 0)).astype(BF16)
    identb = np.eye(128).astype(BF16)
    ones = np.ones((128, 1), np.float32)
    onesb = np.ones((128, 1)).astype(BF16)
    epsb_np = np.full((128, 1), EPS, np.float32)
    xT_bf = xT.astype(BF16)

    mask_list = []
    for tcc, tiles in enumerate(band):
        for (st, full, m) in tiles:
            if not full:
                mask_list.append(m)
    if not mask_list:
        mask_list = [np.zeros((128, CH), np.float32)]
    masks = np.ascontiguousarray(np.stack(mask_list, 0)).astype(BF16)

    in_maps = []
    for c in range(CORES):
        g0 = c * HPC
        rows = []
        for blk in range(3):                                # q, k, v blocks
            for h in range(HPC):
                g = g0 + h
                rows.append(w_qkv[blk * D + g * HD:blk * D + (g + 1) * HD])
        wqkvT = np.ascontiguousarray(np.concatenate(rows, 0).T).astype(BF16)
        woT = np.ascontiguousarray(w_o[c * DSH:(c + 1) * DSH].T).astype(BF16)
        wfcT = np.ascontiguousarray(c_fc[c * FSH:(c + 1) * FSH].T).astype(BF16)
        wpr = np.ascontiguousarray(c_proj[c * FSH:(c + 1) * FSH]).astype(BF16)
        wgT = np.zeros((D, 128), np.float32)
        wgT[:, 0] = ve_gate_w[g0]
        wgT[:, 32] = ve_gate_w[g0 + 1]
        wgT[:, 64] = attn_gate_w[g0]
        wgT[:, 96] = attn_gate_w[g0 + 1]
        wgT = wgT.astype(BF16)
        veS = np.ascontiguousarray(
            ve[0][:, g0 * HD:(g0 + HPC) * HD]).astype(BF16)         # [T, E]
        xTs = np.ascontiguousarray(xT[c * DSH:(c + 1) * DSH])       # [DSH, T]
        in_maps.append(dict(
            xT=xT_bf, xTs=xTs, wqkvT=wqkvT, woT=woT, wfcT=wfcT, wpr=wpr,
            wgT=wgT, veS=veS, ropeA=ropeA, ropeB=ropeB, identb=identb,
            ones=ones, onesb=onesb, epsb=epsb_np, masks=masks))

    res = run_bass_kernel_spmd(nc, in_maps, core_ids=list(range(CORES)))
    LAST_RESULT = res

    outT = res.results[0]["mlp_partialT"].astype(np.float64)
    for c in range(1, CORES):
        outT += res.results[c]["mlp_partialT"]
    for c in range(CORES):
        outT[c * DSH:(c + 1) * DSH] += res.results[c]["x1T_shard"]
    return np.ascontiguousarray(outT.T).astype(np.float32).reshape(1, T, D)


# revision 17
# speedup vs baseline: 1.1222x; 1.1222x over previous
import sys
if '/opt/trn_rl_repo' not in sys.path:
    sys.path.insert(0, '/opt/trn_rl_repo')
import numpy as np
import concourse.bacc as bacc
import concourse.mybir as mybir
import concourse.tile as tile
from concourse.bass_utils import run_bass_kernel_spmd

dt = mybir.dt
AF = mybir.ActivationFunctionType
OP = mybir.AluOpType
BF16 = dt.np(dt.bfloat16)

# problem shapes (hardcoded per spec)
T = 2048
D = 2048
H = 16
HD = 128
FFN = 8192
CORES = 8
HPC = H // CORES          # heads per core = 2
E = HPC * HD              # per-core attention feature width = 256
DSH = D // CORES          # d-shard rows per core = 256
FSH = FFN // CORES        # ffn rows per core = 1024
CH = 512                  # t-chunk
EPS = float(np.finfo(np.float32).eps)

_CACHE = {}
LAST_RESULT = None


def _attn_band(docs):
    """Per t-chunk list of (st, full, mask[128,CH] or None)."""
    docs = np.asarray(docs).astype(np.int64)
    is_sorted = bool(np.all(np.diff(docs) >= 0))
    out = []
    for tc in range(T // CH):
        t0, t1 = tc * CH, (tc + 1) * CH
        if is_sorted:
            s_lo = int(np.searchsorted(docs, docs[t0], side='left'))
            st_lo = s_lo // 128
        else:
            st_lo = 0
        st_hi = (t1 - 1) // 128
        tiles = []
        for st in range(st_lo, st_hi + 1):
            s0, s1 = st * 128, (st + 1) * 128
            m = (np.arange(t0, t1)[None, :] >= np.arange(s0, s1)[:, None]) \
                & (docs[None, t0:t1] == docs[s0:s1, None])
            if not m.any():
                continue
            full = bool(m.all())
            tiles.append((st, full, None if full else m.astype(np.float32)))
        out.append(tiles)
    return out


def _build(band, scale, key_offset):
    """Build the SPMD Bass program (identical for all cores)."""
    nc = bacc.Bacc("TRN2", target_bir_lowering=False, debug=False,
                   num_devices=CORES)
    f32, f32r, bf = dt.float32, dt.float32r, dt.bfloat16

    mask_idx = {}
    for tcc, tiles in enumerate(band):
        for (st, full, m) in tiles:
            if not full:
                mask_idx[(tcc, st)] = len(mask_idx)
    n_masks = max(len(mask_idx), 1)

    # ---- DRAM I/O ----
    xT_d   = nc.dram_tensor("xT",   [D, T], bf, kind="ExternalInput")
    xTs_d  = nc.dram_tensor("xTs",  [DSH, T], f32, kind="ExternalInput")
    wqkv_d = nc.dram_tensor("wqkvT", [D, 3 * E], bf, kind="ExternalInput")
    wo_d   = nc.dram_tensor("woT",  [D, DSH], bf, kind="ExternalInput")
    wfc_d  = nc.dram_tensor("wfcT", [D, FSH], bf, kind="ExternalInput")
    wpr_d  = nc.dram_tensor("wpr",  [FSH, D], bf, kind="ExternalInput")
    wg_d   = nc.dram_tensor("wgT",  [D, 128], bf, kind="ExternalInput")
    ve_d   = nc.dram_tensor("veS",  [T, E], bf, kind="ExternalInput")
    ropeA_d = nc.dram_tensor("ropeA", [HD, T], bf, kind="ExternalInput")
    ropeB_d = nc.dram_tensor("ropeB", [HD, T], bf, kind="ExternalInput")
    idb_d  = nc.dram_tensor("identb", [128, 128], bf, kind="ExternalInput")
    ones_d = nc.dram_tensor("ones", [128, 1], f32r, kind="ExternalInput")
    onesb_d = nc.dram_tensor("onesb", [128, 1], bf, kind="ExternalInput")
    eps_d  = nc.dram_tensor("epsb", [128, 1], f32, kind="ExternalInput")
    mask_d = nc.dram_tensor("masks", [n_masks, 128, CH], bf, kind="ExternalInput")

    mlp_d  = nc.dram_tensor("mlp_partialT", [D, T], f32, kind="ExternalOutput")
    x1s_d  = nc.dram_tensor("x1T_shard", [DSH, T], f32, kind="ExternalOutput")

    NT = T // CH
    ND = D // 128
    NE = DSH // 128
    NF = FSH // 128
    q_of, k_of, v_of = 0, HPC, 2 * HPC

    with tile.TileContext(nc) as tc_:
        with tc_.tile_pool(name="const", bufs=1) as const, \
             tc_.tile_pool(name="dram", bufs=1, space="DRAM") as dram, \
             tc_.tile_pool(name="sDW", bufs=1) as sDW:
            identb = const.tile([128, 128], bf)
            nc.sync.dma_start(identb[:], idb_d.ap())
            ones = const.tile([128, 1], f32r)
            nc.sync.dma_start(ones[:], ones_d.ap())
            onesb = const.tile([128, 1], bf)
            nc.sync.dma_start(onesb[:], onesb_d.ap())
            epsb = const.tile([128, 1], f32)
            nc.sync.dma_start(epsb[:], eps_d.ap())

            # MLP weights prefetched early (DMAs issued in stage A below)
            wfc = sDW.tile([128, ND, FSH], bf)
            wpr = sDW.tile([128, NF, D], bf)

            cc_y_in = [dram.tile([E, CH], bf, name=f"ccyi{t}")
                       for t in range(NT)]
            cc_y_out = [dram.tile([CORES * E, CH], bf, addr_space="Shared",
                                  name=f"ccyo{t}") for t in range(NT)]
            cc_n_in = [dram.tile([1, CH], f32, name=f"ccni{t}")
                       for t in range(NT)]
            cc_n_out = [dram.tile([1, CH], f32, addr_space="Shared",
                                  name=f"ccno{t}") for t in range(NT)]
            cc_x_in = [dram.tile([DSH, CH], bf, name=f"ccxi{t}")
                       for t in range(NT)]
            cc_x_out = [dram.tile([CORES * DSH, CH], bf, addr_space="Shared",
                                  name=f"ccxo{t}") for t in range(NT)]

            with tc_.tile_pool(name="pBC", bufs=1) as pBC:
                x1T = pBC.tile([128, NE, T], f32)

                with tc_.tile_pool(name="pAB", bufs=1) as pAB:
                    qkvT = pAB.tile([128, 3 * HPC, T], bf)
                    # gate rows at partitions 0,32 (ve) and 64,96 (attn)
                    gates = pAB.tile([128, T], bf)

                    # ===== Stage A: qkv + gates on raw x; rms-norm commutes
                    # with the projection, folded into psum eviction =====
                    with tc_.tile_pool(name="stA", bufs=1) as sA, \
                         tc_.tile_pool(name="stA2", bufs=2) as sA2, \
                         tc_.tile_pool(name="psA", bufs=4, space="PSUM") as psA, \
                         tc_.tile_pool(name="psRow", bufs=2, space="PSUM") as psRow:
                        wq = sA.tile([128, ND, 3 * E], bf)
                        nc.sync.dma_start(
                            wq[:], wqkv_d.ap().rearrange("(n p) e -> p n e", p=128))
                        wg = sA.tile([128, ND, 128], bf)
                        nc.sync.dma_start(
                            wg[:], wg_d.ap().rearrange("(n p) e -> p n e", p=128))
                        nc.sync.dma_start(
                            wfc[:], wfc_d.ap().rearrange("(n p) f -> p n f", p=128))
                        nc.sync.dma_start(
                            wpr[:], wpr_d.ap().rearrange("(n p) d -> p n d", p=128))
                        xTr = xT_d.ap().rearrange("(n p) t -> p n t", p=128)
                        for t in range(NT):
                            ts = slice(t * CH, (t + 1) * CH)
                            xt = sA2.tile([128, ND, CH], bf, tag="xt")
                            nc.sync.dma_start(xt[:], xTr[:, :, ts])
                            pr = psRow.tile([1, CH], f32, tag="row")
                            for n in range(ND):
                                sq = sA2.tile([128, CH], f32r, tag="sq")
                                nc.scalar.activation(sq[:], xt[:, n], AF.Square)
                                nc.tensor.matmul(pr[:], ones[:], sq[:],
                                                 start=(n == 0), stop=(n == ND - 1))
                            sd = sA2.tile([1, CH], f32, tag="sd")
                            nc.scalar.activation(sd[:], pr[:], AF.Sqrt,
                                                 bias=epsb[0:1, :], scale=1.0 / D)
                            rcp = sA2.tile([1, CH], f32, tag="rcp")
                            nc.vector.reciprocal_approx_fast(rcp[:], sd[:])
                            rep = sA2.tile([128, CH], f32, tag="rep")
                            nc.gpsimd.partition_broadcast(rep[:], rcp[:])
                            for m in range(3 * HPC):
                                pq = psA.tile([128, CH], f32, tag="mm")
                                for n in range(ND):
                                    nc.tensor.matmul(
                                        pq[:], wq[:, n, m * 128:(m + 1) * 128],
                                        xt[:, n], start=(n == 0), stop=(n == ND - 1))
                                nc.vector.tensor_tensor(qkvT[:, m, ts], pq[:],
                                                        rep[:], OP.mult)
                            pg = psA.tile([128, CH], f32, tag="g", bufs=2)
                            for n in range(ND):
                                nc.tensor.matmul(pg[:], wg[:, n], xt[:, n],
                                                 start=(n == 0), stop=(n == ND - 1))
                            gm = sA2.tile([128, CH], f32, tag="gm")
                            nc.vector.tensor_tensor(gm[:], pg[:], rep[:], OP.mult)
                            nc.scalar.activation(gates[:, ts], gm[:], AF.Sigmoid)

                    # ===== Stage B: attention; y AllGather chunked by t =====
                    with tc_.tile_pool(name="stB", bufs=1) as sB, \
                         tc_.tile_pool(name="stB2", bufs=2) as sB2, \
                         tc_.tile_pool(name="psB", bufs=2, space="PSUM") as psB, \
                         tc_.tile_pool(name="psBa", bufs=2, space="PSUM") as psBa, \
                         tc_.tile_pool(name="psBr", bufs=2, space="PSUM") as psBr:
                        ropeA = sB.tile([HD, T], bf)
                        nc.sync.dma_start(ropeA[:], ropeA_d.ap())
                        ropeB = sB.tile([HD, T], bf)
                        nc.sync.dma_start(ropeB[:], ropeB_d.ap())
                        gcol = sB.tile([128, T // 128, 2], bf)
                        for st in range(T // 128):
                            pt = psB.tile([128, 128], bf, tag="vt")
                            nc.tensor.transpose(
                                pt[:], gates[:, st * 128:(st + 1) * 128], identb[:])
                            nc.vector.tensor_copy(gcol[:, st, 0:1], pt[:, 0:1])
                            nc.vector.tensor_copy(gcol[:, st, 1:2], pt[:, 32:33])

                        qrs, kfs, vuses = [], [], []
                        for h in range(HPC):
                            q = qkvT[:, q_of + h]
                            k = qkvT[:, k_of + h]
                            vt = qkvT[:, v_of + h]
                            hh = HD // 2
                            qr = sB.tile([128, T], bf, tag=f"qr{h}")
                            kr = sB.tile([128, T], bf, tag="kr")
                            tmp = sB.tile([128, T], bf, tag="tmp")
                            # rotary on raw q/k (rms-norm folded in after:
                            # rot(n*q) = n*rot(q)). ropeA = [cos;sin],
                            # ropeB = [sin;cos] for base-partition alignment.
                            for (src_, dst) in ((q, qr), (k, kr)):
                                x1, x2 = src_[0:hh, :], src_[hh:HD, :]
                                cosA, sinA = ropeA[0:hh, :], ropeA[hh:HD, :]
                                sinB, cosB = ropeB[0:hh, :], ropeB[hh:HD, :]
                                nc.vector.tensor_tensor(tmp[0:hh, :], x2, sinA,
                                                        OP.mult)
                                nc.vector.tensor_tensor(dst[0:hh, :], x1, cosA,
                                                        OP.mult)
                                nc.vector.tensor_tensor(dst[0:hh, :], dst[0:hh, :],
                                                        tmp[0:hh, :], OP.add)
                                nc.vector.tensor_tensor(tmp[hh:HD, :], x1, sinB,
                                                        OP.mult)
                                nc.vector.tensor_tensor(dst[hh:HD, :], x2, cosB,
                                                        OP.mult)
                                nc.vector.tensor_tensor(dst[hh:HD, :], dst[hh:HD, :],
                                                        tmp[hh:HD, :], OP.subtract)
                                # rms-norm factors (from raw src) applied to dst
                                for t in range(NT):
                                    ts = slice(t * CH, (t + 1) * CH)
                                    sq2 = sB2.tile([128, CH], f32r, tag="sq2")
                                    nc.scalar.activation(sq2[:], src_[:, ts],
                                                         AF.Square)
                                    pr2 = psBr.tile([1, CH], f32, tag="row")
                                    nc.tensor.matmul(pr2[:], ones[:], sq2[:],
                                                     start=True, stop=True)
                                    sd2 = sB2.tile([1, CH], f32, tag="sd2")
                                    nc.scalar.activation(sd2[:], pr2[:], AF.Sqrt,
                                                         bias=epsb[0:1, :],
                                                         scale=1.0 / HD)
                                    rcp2 = sB2.tile([1, CH], f32, tag="rcp2")
                                    nc.vector.reciprocal_approx_fast(rcp2[:], sd2[:])
                                    rep2 = sB2.tile([128, CH], f32, tag="rep2")
                                    nc.gpsimd.partition_broadcast(rep2[:], rcp2[:])
                                    nc.vector.tensor_tensor(dst[:, ts], dst[:, ts],
                                                            rep2[:], OP.mult)
                            # key_offset shift (copies on GpSimd, 1-input ops
                            # run at line rate there and DVE is the hot engine)
                            if key_offset:
                                kf = sB.tile([128, T], bf, tag=f"kf{h}")
                                a, b, c3 = HD // 4, HD // 2, 3 * HD // 4
                                nc.vector.tensor_copy(kf[0:a, :], kr[0:a, :])
                                nc.vector.tensor_copy(kf[b:c3, :], kr[b:c3, :])
                                nc.vector.tensor_copy(kf[a:b, 1:T], kr[a:b, 0:T - 1])
                                nc.vector.tensor_copy(kf[a:b, 0:1], kr[a:b, 0:1])
                                nc.vector.tensor_copy(kf[c3:HD, 1:T],
                                                      kr[c3:HD, 0:T - 1])
                                nc.vector.tensor_copy(kf[c3:HD, 0:1], kr[c3:HD, 0:1])
                            else:
                                kf = sB.tile([128, T], bf, tag=f"kf{h}")
                                nc.vector.tensor_copy(kf[:], kr[:])
                            # v natural + ve gate
                            ve_nat = sB.tile([128, T // 128, HD], bf, tag="ve")
                            nc.sync.dma_start(
                                ve_nat[:],
                                ve_d.ap()[:, h * HD:(h + 1) * HD]
                                    .rearrange("(n p) e -> p n e", p=128))
                            vuse = sB.tile([128, T // 128, HD], bf, tag=f"vuse{h}")
                            for st in range(T // 128):
                                pv = psB.tile([128, HD], bf, tag="vt")
                                nc.tensor.transpose(
                                    pv[:], vt[:, st * 128:(st + 1) * 128], identb[:])
                                nc.vector.scalar_tensor_tensor(
                                    vuse[:, st], ve_nat[:, st],
                                    gcol[:, st, h:h + 1], pv[:], OP.mult, OP.add)
                            qrs.append(qr)
                            kfs.append(kf)
                            vuses.append(vuse)

                        # attention over the same-doc causal band, t outer so
                        # each finished chunk AllGathers immediately
                        yTt = None
                        for t in range(NT):
                            ts = slice(t * CH, (t + 1) * CH)
                            tiles = band[t]
                            yTt = sB.tile([128, HPC, CH], bf, tag="yTt", bufs=2)
                            for h in range(HPC):
                                qr, kf, vuse = qrs[h], kfs[h], vuses[h]
                                py = psBa.tile([128, CH], f32, tag="y")
                                pden = psBr.tile([1, CH], f32, tag="row")
                                for i, (st, full, _m) in enumerate(tiles):
                                    ps = psB.tile([128, CH], f32, tag="s")
                                    nc.tensor.matmul(
                                        ps[:], kf[:, st * 128:(st + 1) * 128],
                                        qr[:, ts], start=True, stop=True)
                                    pT = sB2.tile([128, CH], bf, tag="pT")
                                    nc.scalar.activation(pT[:], ps[:], AF.Exp,
                                                         scale=scale)
                                    if not full:
                                        mk = sB2.tile([128, CH], bf, tag="mk")
                                        nc.sync.dma_start(
                                            mk[:], mask_d.ap()[mask_idx[(t, st)]])
                                        nc.vector.tensor_tensor(pT[:], pT[:], mk[:],
                                                                OP.mult)
                                    nc.tensor.matmul(
                                        py[:], vuse[:, st], pT[:],
                                        start=(i == 0), stop=(i == len(tiles) - 1))
                                    nc.tensor.matmul(
                                        pden[:], onesb[:], pT[:],
                                        start=(i == 0), stop=(i == len(tiles) - 1))
                                rec = sB2.tile([1, CH], f32, tag="rec")
                                nc.vector.reciprocal_approx_fast(rec[:], pden[:])
                                grow = sB2.tile([1, CH], f32, tag="grow")
                                nc.vector.tensor_copy(
                                    grow[:], gates[64 + 32 * h:65 + 32 * h, ts])
                                comb = sB2.tile([1, CH], f32, tag="comb")
                                nc.vector.tensor_tensor(comb[:], rec[:], grow[:],
                                                        OP.mult)
                                repy = sB2.tile([128, CH], f32, tag="repy")
                                nc.gpsimd.partition_broadcast(repy[:], comb[:])
                                nc.vector.tensor_tensor(yTt[:, h], py[:],
                                                        repy[:], OP.mult)
                            nc.sync.dma_start(
                                cc_y_in[t][:].rearrange("(h p) c -> p h c", p=128),
                                yTt[:])
                            nc.gpsimd.collective_compute(
                                "AllGather", OP.bypass,
                                replica_groups=[list(range(CORES))],
                                ins=[cc_y_in[t][:].opt()],
                                outs=[cc_y_out[t][:].opt()])

                # ===== Stage C: o-proj per chunk, norm, chunked AG(xn1) =====
                with tc_.tile_pool(name="stC", bufs=1) as sC, \
                     tc_.tile_pool(name="stC2", bufs=2) as sC2, \
                     tc_.tile_pool(name="psC", bufs=2, space="PSUM") as psC, \
                     tc_.tile_pool(name="psCr", bufs=2, space="PSUM") as psCr:
                    wo = sC.tile([128, ND, DSH], bf)
                    nc.sync.dma_start(
                        wo[:], wo_d.ap().rearrange("(n p) e -> p n e", p=128))
                    xs = sC.tile([128, NE, T], f32)
                    nc.sync.dma_start(
                        xs[:], xTs_d.ap().rearrange("(n p) t -> p n t", p=128))
                    nsum = sC.tile([1, T], f32)
                    for t in range(NT):
                        ts = slice(t * CH, (t + 1) * CH)
                        ygt = sC2.tile([128, ND, CH], bf, tag="ygt")
                        nc.sync.dma_start(
                            ygt[:],
                            cc_y_out[t][:].rearrange("(n p) c -> p n c", p=128))
                        for m in range(NE):
                            po = psC.tile([128, CH], f32, tag="o")
                            for n in range(ND):
                                nc.tensor.matmul(
                                    po[:], wo[:, n, m * 128:(m + 1) * 128],
                                    ygt[:, n], start=(n == 0), stop=(n == ND - 1))
                            nc.vector.tensor_tensor(x1T[:, m, ts], po[:],
                                                    xs[:, m, ts], OP.add)
                        pr3 = psCr.tile([1, CH], f32, tag="r3")
                        for m in range(NE):
                            sq3 = sC2.tile([128, CH], f32r, tag="sq3")
                            nc.scalar.activation(sq3[:], x1T[:, m, ts], AF.Square)
                            nc.tensor.matmul(pr3[:], ones[:], sq3[:],
                                             start=(m == 0), stop=(m == NE - 1))
                        nc.vector.tensor_copy(nsum[:, ts], pr3[:])
                        nc.sync.dma_start(cc_n_in[t][:], nsum[:, ts])
                        nc.gpsimd.collective_compute(
                            "AllReduce", OP.add,
                            replica_groups=[list(range(CORES))],
                            ins=[cc_n_in[t][:].opt()], outs=[cc_n_out[t][:].opt()])
                        nfull = sC2.tile([1, CH], f32, tag="nfull")
                        nc.sync.dma_start(nfull[:], cc_n_out[t][:])
                        sd3 = sC2.tile([1, CH], f32, tag="sd3")
                        nc.scalar.activation(sd3[:], nfull[:], AF.Sqrt,
                                             bias=epsb[0:1, :], scale=1.0 / D)
                        rcp3 = sC2.tile([1, CH], f32, tag="rcp3")
                        nc.vector.reciprocal_approx_fast(rcp3[:], sd3[:])
                        rep3 = sC2.tile([128, CH], f32, tag="rep3")
                        nc.gpsimd.partition_broadcast(rep3[:], rcp3[:])
                        xn1 = sC2.tile([128, NE, CH], bf, tag="xn1")
                        for m in range(NE):
                            nc.vector.tensor_tensor(xn1[:, m], x1T[:, m, ts],
                                                    rep3[:], OP.mult)
                        nc.sync.dma_start(
                            cc_x_in[t][:].rearrange("(n p) c -> p n c", p=128),
                            xn1[:])
                        nc.gpsimd.collective_compute(
                            "AllGather", OP.bypass,
                            replica_groups=[list(range(CORES))],
                            ins=[cc_x_in[t][:].opt()], outs=[cc_x_out[t][:].opt()])
                    nc.sync.dma_start(
                        x1s_d.ap().rearrange("(n p) t -> p n t", p=128), x1T[:])

            # ===== Stage D: MLP (weights prefetched in sDW) =====
            with tc_.tile_pool(name="stD2", bufs=2) as sD2, \
                 tc_.tile_pool(name="psD", bufs=3, space="PSUM") as psD:
                mlpr = mlp_d.ap().rearrange("(n p) t -> p n t", p=128)
                for t in range(NT):
                    ts = slice(t * CH, (t + 1) * CH)
                    xnt = sD2.tile([128, ND, CH], bf, tag="xnt")
                    nc.sync.dma_start(
                        xnt[:],
                        cc_x_out[t][:].rearrange("(n p) c -> p n c", p=128))
                    hT = sD2.tile([128, NF, CH], bf, tag="hT")
                    for mf in range(NF):
                        ph = psD.tile([128, CH], f32, tag="h")
                        for n in range(ND):
                            nc.tensor.matmul(
                                ph[:], wfc[:, n, mf * 128:(mf + 1) * 128],
                                xnt[:, n], start=(n == 0), stop=(n == ND - 1))
                        hr = sD2.tile([128, CH], f32, tag="hr")
                        nc.scalar.activation(hr[:], ph[:], AF.Relu)
                        nc.vector.tensor_tensor(hT[:, mf], hr[:], hr[:], OP.mult)
                    for md in range(ND):
                        pm = psD.tile([128, CH], f32, tag="m")
                        for mf in range(NF):
                            nc.tensor.matmul(
                                pm[:], wpr[:, mf, md * 128:(md + 1) * 128],
                                hT[:, mf], start=(mf == 0), stop=(mf == NF - 1))
                        ot = sD2.tile([128, CH], f32, tag="ot")
                        nc.vector.tensor_copy(ot[:], pm[:])
                        nc.sync.dma_start(mlpr[:, md, ts], ot[:])

    nc.compile()
    return nc


def kernel(x, ve, qkvo_w, attn_gate_w, ve_gate_w, c_fc, c_proj,
           sa_lambdas, cos, sin, attn_scale, docs, key_offset):
    global LAST_RESULT
    x = np.asarray(x, np.float32)
    ve = np.asarray(ve, np.float32)
    qkvo_w = np.asarray(qkvo_w, np.float32)
    attn_gate_w = np.asarray(attn_gate_w, np.float32)
    ve_gate_w = np.asarray(ve_gate_w, np.float32)
    c_fc = np.asarray(c_fc, np.float32)
    c_proj = np.asarray(c_proj, np.float32)
    sa = np.asarray(sa_lambdas, np.float32)
    docs = np.asarray(docs)
    ko = int(np.asarray(key_offset))
    scale = float(np.asarray(attn_scale).reshape(-1)[0])

    band = _attn_band(docs)
    key = (docs.tobytes(), scale, ko)
    if key not in _CACHE:
        _CACHE[key] = _build(band, scale, ko)
    nc = _CACHE[key]

    xT = np.ascontiguousarray(x[0].T)                       # [D, T] f32
    w_qkv = sa[0] * qkvo_w[:3 * D]                          # [3D, D]
    w_o = sa[1] * qkvo_w[3 * D:]                            # [D, D]
    cosT = np.asarray(cos, np.float32).T
    sinT = np.asarray(sin, np.float32).T
    ropeA = np.ascontiguousarray(np.concatenate([cosT, sinT], 0)).astype(BF16)
    ropeB = np.ascontiguousarray(np.concatenate([sinT, cosT], 0)).astype(BF16)
    identb = np.eye(128).astype(BF16)
    ones = np.ones((128, 1), np.float32)
    onesb = np.ones((128, 1)).astype(BF16)
    epsb_np = np.full((128, 1), EPS, np.float32)
    xT_bf = xT.astype(BF16)

    mask_list = []
    for tcc, tiles in enumerate(band):
        for (st, full, m) in tiles:
            if not full:
                mask_list.append(m)
    if not mask_list:
        mask_list = [np.zeros((128, CH), np.float32)]
    masks = np.ascontiguousarray(np.stack(mask_list, 0)).astype(BF16)

    in_maps = []
    for c in range(CORES):
        g0 = c * HPC
        rows = []
        for blk in range(3):                                # q, k, v blocks
            for h in range(HPC):
                g = g0 + h
                rows.append(w_qkv[blk * D + g * HD:blk * D + (g + 1) * HD])
        wqkvT = np.ascontiguousarray(np.concatenate(rows, 0).T).astype(BF16)
        woT = np.ascontiguousarray(w_o[c * DSH:(c + 1) * DSH].T).astype(BF16)
        wfcT = np.ascontiguousarray(c_fc[c * FSH:(c + 1) * FSH].T).astype(BF16)
        wpr = np.ascontiguousarray(c_proj[c * FSH:(c + 1) * FSH]).astype(BF16)
        wgT = np.zeros((D, 128), np.float32)
        wgT[:, 0] = ve_gate_w[g0]
        wgT[:, 32] = ve_gate_w[g0 + 1]
        wgT[:, 64] = attn_gate_w[g0]
        wgT[:, 96] = attn_gate_w[g0 + 1]
        wgT = wgT.astype(BF16)
        veS = np.ascontiguousarray(
            ve[0][:, g0 * HD:(g0 + HPC) * HD]).astype(BF16)         # [T, E]
        xTs = np.ascontiguousarray(xT[c * DSH:(c + 1) * DSH])       # [DSH, T]
        in_maps.append(dict(
            xT=xT_bf, xTs=xTs, wqkvT=wqkvT, woT=woT, wfcT=wfcT, wpr=wpr,
            wgT=wgT, veS=veS, ropeA=ropeA, ropeB=ropeB, identb=identb,
            ones=ones, onesb=onesb, epsb=epsb_np, masks=masks))

    res = run_bass_kernel_spmd(nc, in_maps, core_ids=list(range(CORES)))
    LAST_RESULT = res

    outT = res.results[0]["mlp_partialT"].astype(np.float64)
    for c in range(1, CORES):
        outT += res.results[c]["mlp_partialT"]
    for c in range(CORES):
        outT[c * DSH:(c + 1) * DSH] += res.results[c]["x1T_shard"]
    return np.ascontiguousarray(outT.T).astype(np.float32).reshape(1, T, D)


# revision 18
# speedup vs baseline: 1.1832x; 1.0543x over previous
import sys
if '/opt/trn_rl_repo' not in sys.path:
    sys.path.insert(0, '/opt/trn_rl_repo')
import numpy as np
import concourse.bacc as bacc
import concourse.mybir as mybir
import concourse.tile as tile
from concourse.bass_utils import run_bass_kernel_spmd

dt = mybir.dt
AF = mybir.ActivationFunctionType
OP = mybir.AluOpType
BF16 = dt.np(dt.bfloat16)

# problem shapes (hardcoded per spec)
T = 2048
D = 2048
H = 16
HD = 128
FFN = 8192
CORES = 8
HPC = H // CORES          # heads per core = 2
E = HPC * HD              # per-core attention feature width = 256
DSH = D // CORES          # d-shard rows per core = 256
FSH = FFN // CORES        # ffn rows per core = 1024
CH = 512                  # t-chunk
EPS = float(np.finfo(np.float32).eps)

_CACHE = {}
LAST_RESULT = None


def _attn_band(docs):
    """Per t-chunk list of (st, full, mask[128,CH] or None)."""
    docs = np.asarray(docs).astype(np.int64)
    is_sorted = bool(np.all(np.diff(docs) >= 0))
    out = []
    for tc in range(T // CH):
        t0, t1 = tc * CH, (tc + 1) * CH
        if is_sorted:
            s_lo = int(np.searchsorted(docs, docs[t0], side='left'))
            st_lo = s_lo // 128
        else:
            st_lo = 0
        st_hi = (t1 - 1) // 128
        tiles = []
        for st in range(st_lo, st_hi + 1):
            s0, s1 = st * 128, (st + 1) * 128
            m = (np.arange(t0, t1)[None, :] >= np.arange(s0, s1)[:, None]) \
                & (docs[None, t0:t1] == docs[s0:s1, None])
            if not m.any():
                continue
            full = bool(m.all())
            tiles.append((st, full, None if full else m.astype(np.float32)))
        out.append(tiles)
    return out


def _build(band, scale, key_offset):
    """Build the SPMD Bass program (identical for all cores)."""
    nc = bacc.Bacc("TRN2", target_bir_lowering=False, debug=False,
                   num_devices=CORES)
    f32, f32r, bf = dt.float32, dt.float32r, dt.bfloat16

    mask_idx = {}
    for tcc, tiles in enumerate(band):
        for (st, full, m) in tiles:
            if not full:
                mask_idx[(tcc, st)] = len(mask_idx)
    n_masks = max(len(mask_idx), 1)

    # ---- DRAM I/O ----
    xT_d   = nc.dram_tensor("xT",   [D, T], bf, kind="ExternalInput")
    xTs_d  = nc.dram_tensor("xTs",  [DSH, T], f32, kind="ExternalInput")
    wqkv_d = nc.dram_tensor("wqkvT", [D, 3 * E], bf, kind="ExternalInput")
    wo_d   = nc.dram_tensor("woT",  [D, DSH], bf, kind="ExternalInput")
    wfc_d  = nc.dram_tensor("wfcT", [D, FSH], bf, kind="ExternalInput")
    wpr_d  = nc.dram_tensor("wpr",  [FSH, D], bf, kind="ExternalInput")
    wg_d   = nc.dram_tensor("wgT",  [D, 128], bf, kind="ExternalInput")
    ve_d   = nc.dram_tensor("veS",  [T, E], bf, kind="ExternalInput")
    ropeA_d = nc.dram_tensor("ropeA", [HD, T], bf, kind="ExternalInput")
    ropeB_d = nc.dram_tensor("ropeB", [HD, T], bf, kind="ExternalInput")
    idb_d  = nc.dram_tensor("identb", [128, 128], bf, kind="ExternalInput")
    ones_d = nc.dram_tensor("ones", [128, 1], f32r, kind="ExternalInput")
    onesb_d = nc.dram_tensor("onesb", [128, 1], bf, kind="ExternalInput")
    eps_d  = nc.dram_tensor("epsb", [128, 1], f32, kind="ExternalInput")
    mask_d = nc.dram_tensor("masks", [n_masks, 128, CH], bf, kind="ExternalInput")

    mlp_d  = nc.dram_tensor("mlp_partialT", [D, T], f32, kind="ExternalOutput")
    x1s_d  = nc.dram_tensor("x1T_shard", [DSH, T], f32, kind="ExternalOutput")

    NT = T // CH
    ND = D // 128
    NE = DSH // 128
    NF = FSH // 128
    q_of, k_of, v_of = 0, HPC, 2 * HPC

    with tile.TileContext(nc) as tc_:
        with tc_.tile_pool(name="const", bufs=1) as const, \
             tc_.tile_pool(name="dram", bufs=1, space="DRAM") as dram, \
             tc_.tile_pool(name="sDW", bufs=1) as sDW:
            identb = const.tile([128, 128], bf)
            nc.sync.dma_start(identb[:], idb_d.ap())
            ones = const.tile([128, 1], f32r)
            nc.sync.dma_start(ones[:], ones_d.ap())
            onesb = const.tile([128, 1], bf)
            nc.sync.dma_start(onesb[:], onesb_d.ap())
            epsb = const.tile([128, 1], f32)
            nc.sync.dma_start(epsb[:], eps_d.ap())

            # stage C/D weights prefetched early (DMAs issued in stage B)
            wfc = sDW.tile([128, ND, FSH], bf)
            wpr = sDW.tile([128, NF, D], bf)
            wo = sDW.tile([128, ND, DSH], bf)

            cc_y_in = [dram.tile([E, CH], bf, name=f"ccyi{t}")
                       for t in range(NT)]
            cc_y_out = [dram.tile([CORES * E, CH], bf, addr_space="Shared",
                                  name=f"ccyo{t}") for t in range(NT)]
            cc_n_in = [dram.tile([1, CH], f32, name=f"ccni{t}")
                       for t in range(NT)]
            cc_n_out = [dram.tile([1, CH], f32, addr_space="Shared",
                                  name=f"ccno{t}") for t in range(NT)]
            cc_x_in = [dram.tile([DSH, CH], bf, name=f"ccxi{t}")
                       for t in range(NT)]
            cc_x_out = [dram.tile([CORES * DSH, CH], bf, addr_space="Shared",
                                  name=f"ccxo{t}") for t in range(NT)]

            with tc_.tile_pool(name="pBC", bufs=1) as pBC:
                x1T = pBC.tile([128, NE, T], f32)

                with tc_.tile_pool(name="pAB", bufs=1) as pAB:
                    qkvT = pAB.tile([128, 3 * HPC, T], bf)
                    # gate rows at partitions 0,32 (ve) and 64,96 (attn)
                    gates = pAB.tile([128, T], bf)

                    # ===== Stage A: qkv + gates on raw x; rms-norm commutes
                    # with the projection, folded into psum eviction =====
                    with tc_.tile_pool(name="stA", bufs=1) as sA, \
                         tc_.tile_pool(name="stA2", bufs=2) as sA2, \
                         tc_.tile_pool(name="psA", bufs=4, space="PSUM") as psA, \
                         tc_.tile_pool(name="psRow", bufs=2, space="PSUM") as psRow:
                        wq = sA.tile([128, ND, 3 * E], bf)
                        nc.sync.dma_start(
                            wq[:], wqkv_d.ap().rearrange("(n p) e -> p n e", p=128))
                        wg = sA.tile([128, ND, 128], bf)
                        nc.sync.dma_start(
                            wg[:], wg_d.ap().rearrange("(n p) e -> p n e", p=128))
                        xTr = xT_d.ap().rearrange("(n p) t -> p n t", p=128)
                        for t in range(NT):
                            ts = slice(t * CH, (t + 1) * CH)
                            xt = sA2.tile([128, ND, CH], bf, tag="xt")
                            nc.sync.dma_start(xt[:], xTr[:, :, ts])
                            pr = psRow.tile([1, CH], f32, tag="row")
                            for n in range(ND):
                                sq = sA2.tile([128, CH], f32r, tag="sq")
                                nc.scalar.activation(sq[:], xt[:, n], AF.Square)
                                nc.tensor.matmul(pr[:], ones[:], sq[:],
                                                 start=(n == 0), stop=(n == ND - 1))
                            sd = sA2.tile([1, CH], f32, tag="sd")
                            nc.scalar.activation(sd[:], pr[:], AF.Sqrt,
                                                 bias=epsb[0:1, :], scale=1.0 / D)
                            rcp = sA2.tile([1, CH], f32, tag="rcp")
                            nc.vector.reciprocal_approx_fast(rcp[:], sd[:])
                            rep = sA2.tile([128, CH], f32, tag="rep")
                            nc.gpsimd.partition_broadcast(rep[:], rcp[:])
                            for m in range(3 * HPC):
                                pq = psA.tile([128, CH], f32, tag="mm")
                                for n in range(ND):
                                    nc.tensor.matmul(
                                        pq[:], wq[:, n, m * 128:(m + 1) * 128],
                                        xt[:, n], start=(n == 0), stop=(n == ND - 1))
                                nc.vector.tensor_tensor(qkvT[:, m, ts], pq[:],
                                                        rep[:], OP.mult)
                            pg = psA.tile([128, CH], f32, tag="g", bufs=2)
                            for n in range(ND):
                                nc.tensor.matmul(pg[:], wg[:, n], xt[:, n],
                                                 start=(n == 0), stop=(n == ND - 1))
                            gm = sA2.tile([128, CH], f32, tag="gm")
                            nc.vector.tensor_tensor(gm[:], pg[:], rep[:], OP.mult)
                            nc.scalar.activation(gates[:, ts], gm[:], AF.Sigmoid)

                    # ===== Stage B: attention; y AllGather chunked by t =====
                    with tc_.tile_pool(name="stB", bufs=1) as sB, \
                         tc_.tile_pool(name="stB2", bufs=2) as sB2, \
                         tc_.tile_pool(name="psB", bufs=2, space="PSUM") as psB, \
                         tc_.tile_pool(name="psBa", bufs=2, space="PSUM") as psBa, \
                         tc_.tile_pool(name="psBr", bufs=2, space="PSUM") as psBr:
                        ropeA = sB.tile([HD, T], bf)
                        nc.sync.dma_start(ropeA[:], ropeA_d.ap())
                        ropeB = sB.tile([HD, T], bf)
                        nc.sync.dma_start(ropeB[:], ropeB_d.ap())
                        nc.sync.dma_start(
                            wo[:], wo_d.ap().rearrange("(n p) e -> p n e", p=128))
                        nc.sync.dma_start(
                            wfc[:], wfc_d.ap().rearrange("(n p) f -> p n f", p=128))
                        nc.sync.dma_start(
                            wpr[:], wpr_d.ap().rearrange("(n p) d -> p n d", p=128))
                        gcol = sB.tile([128, T // 128, 2], bf)
                        for st in range(T // 128):
                            pt = psB.tile([128, 128], bf, tag="vt")
                            nc.tensor.transpose(
                                pt[:], gates[:, st * 128:(st + 1) * 128], identb[:])
                            nc.vector.tensor_copy(gcol[:, st, 0:1], pt[:, 0:1])
                            nc.vector.tensor_copy(gcol[:, st, 1:2], pt[:, 32:33])

                        qrs, kfs, vuses = [], [], []
                        for h in range(HPC):
                            q = qkvT[:, q_of + h]
                            k = qkvT[:, k_of + h]
                            vt = qkvT[:, v_of + h]
                            hh = HD // 2
                            qr = sB.tile([128, T], bf, tag=f"qr{h}")
                            kr = sB.tile([128, T], bf, tag="kr")
                            tmp = sB.tile([128, T], bf, tag="tmp")
                            # rotary on raw q/k (rms-norm folded in after:
                            # rot(n*q) = n*rot(q)). ropeA = [cos;sin],
                            # ropeB = [sin;cos] for base-partition alignment.
                            for (src_, dst) in ((q, qr), (k, kr)):
                                x1, x2 = src_[0:hh, :], src_[hh:HD, :]
                                cosA, sinA = ropeA[0:hh, :], ropeA[hh:HD, :]
                                sinB, cosB = ropeB[0:hh, :], ropeB[hh:HD, :]
                                nc.vector.tensor_tensor(tmp[0:hh, :], x2, sinA,
                                                        OP.mult)
                                nc.vector.tensor_tensor(dst[0:hh, :], x1, cosA,
                                                        OP.mult)
                                nc.vector.tensor_tensor(dst[0:hh, :], dst[0:hh, :],
                                                        tmp[0:hh, :], OP.add)
                                nc.vector.tensor_tensor(tmp[hh:HD, :], x1, sinB,
                                                        OP.mult)
                                nc.vector.tensor_tensor(dst[hh:HD, :], x2, cosB,
                                                        OP.mult)
                                nc.vector.tensor_tensor(dst[hh:HD, :], dst[hh:HD, :],
                                                        tmp[hh:HD, :], OP.subtract)
                                # rms-norm factors (from raw src) applied to dst
                                for t in range(NT):
                                    ts = slice(t * CH, (t + 1) * CH)
                                    sq2 = sB2.tile([128, CH], f32r, tag="sq2")
                                    nc.scalar.activation(sq2[:], src_[:, ts],
                                                         AF.Square)
                                    pr2 = psBr.tile([1, CH], f32, tag="row")
                                    nc.tensor.matmul(pr2[:], ones[:], sq2[:],
                                                     start=True, stop=True)
                                    sd2 = sB2.tile([1, CH], f32, tag="sd2")
                                    nc.scalar.activation(sd2[:], pr2[:], AF.Sqrt,
                                                         bias=epsb[0:1, :],
                                                         scale=1.0 / HD)
                                    rcp2 = sB2.tile([1, CH], f32, tag="rcp2")
                                    nc.vector.reciprocal_approx_fast(rcp2[:], sd2[:])
                                    rep2 = sB2.tile([128, CH], f32, tag="rep2")
                                    nc.gpsimd.partition_broadcast(rep2[:], rcp2[:])
                                    nc.vector.tensor_tensor(dst[:, ts], dst[:, ts],
                                                            rep2[:], OP.mult)
                            # key_offset shift (copies on GpSimd, 1-input ops
                            # run at line rate there and DVE is the hot engine)
                            if key_offset:
                                kf = sB.tile([128, T], bf, tag=f"kf{h}")
                                a, b, c3 = HD // 4, HD // 2, 3 * HD // 4
                                nc.scalar.copy(kf[0:a, :], kr[0:a, :])
                                nc.scalar.copy(kf[b:c3, :], kr[b:c3, :])
                                nc.scalar.copy(kf[a:b, 1:T], kr[a:b, 0:T - 1])
                                nc.scalar.copy(kf[a:b, 0:1], kr[a:b, 0:1])
                                nc.scalar.copy(kf[c3:HD, 1:T],
                                                      kr[c3:HD, 0:T - 1])
                                nc.scalar.copy(kf[c3:HD, 0:1], kr[c3:HD, 0:1])
                            else:
                                kf = sB.tile([128, T], bf, tag=f"kf{h}")
                                nc.scalar.copy(kf[:], kr[:])
                            # v natural + ve gate
                            ve_nat = sB.tile([128, T // 128, HD], bf, tag="ve")
                            nc.sync.dma_start(
                                ve_nat[:],
                                ve_d.ap()[:, h * HD:(h + 1) * HD]
                                    .rearrange("(n p) e -> p n e", p=128))
                            vuse = sB.tile([128, T // 128, HD], bf, tag=f"vuse{h}")
                            for st in range(T // 128):
                                pv = psB.tile([128, HD], bf, tag="vt")
                                nc.tensor.transpose(
                                    pv[:], vt[:, st * 128:(st + 1) * 128], identb[:])
                                nc.vector.scalar_tensor_tensor(
                                    vuse[:, st], ve_nat[:, st],
                                    gcol[:, st, h:h + 1], pv[:], OP.mult, OP.add)
                            qrs.append(qr)
                            kfs.append(kf)
                            vuses.append(vuse)

                        # attention over the same-doc causal band, t outer so
                        # each finished chunk AllGathers immediately
                        yTt = None
                        for t in range(NT):
                            ts = slice(t * CH, (t + 1) * CH)
                            tiles = band[t]
                            yTt = sB.tile([128, HPC, CH], bf, tag="yTt", bufs=2)
                            for h in range(HPC):
                                qr, kf, vuse = qrs[h], kfs[h], vuses[h]
                                py = psBa.tile([128, CH], f32, tag="y")
                                pden = psBr.tile([1, CH], f32, tag="row")
                                for i, (st, full, _m) in enumerate(tiles):
                                    ps = psB.tile([128, CH], f32, tag="s")
                                    nc.tensor.matmul(
                                        ps[:], kf[:, st * 128:(st + 1) * 128],
                                        qr[:, ts], start=True, stop=True)
                                    pT = sB2.tile([128, CH], bf, tag="pT")
                                    nc.scalar.activation(pT[:], ps[:], AF.Exp,
                                                         scale=scale)
                                    if not full:
                                        mk = sB2.tile([128, CH], bf, tag="mk")
                                        nc.sync.dma_start(
                                            mk[:], mask_d.ap()[mask_idx[(t, st)]])
                                        nc.vector.tensor_tensor(pT[:], pT[:], mk[:],
                                                                OP.mult)
                                    nc.tensor.matmul(
                                        py[:], vuse[:, st], pT[:],
                                        start=(i == 0), stop=(i == len(tiles) - 1))
                                    nc.tensor.matmul(
                                        pden[:], onesb[:], pT[:],
                                        start=(i == 0), stop=(i == len(tiles) - 1))
                                rec = sB2.tile([1, CH], f32, tag="rec")
                                nc.vector.reciprocal_approx_fast(rec[:], pden[:])
                                grow = sB2.tile([1, CH], f32, tag="grow")
                                nc.vector.tensor_copy(
                                    grow[:], gates[64 + 32 * h:65 + 32 * h, ts])
                                comb = sB2.tile([1, CH], f32, tag="comb")
                                nc.vector.tensor_tensor(comb[:], rec[:], grow[:],
                                                        OP.mult)
                                repy = sB2.tile([128, CH], f32, tag="repy")
                                nc.gpsimd.partition_broadcast(repy[:], comb[:])
                                nc.vector.tensor_tensor(yTt[:, h], py[:],
                                                        repy[:], OP.mult)
                            nc.sync.dma_start(
                                cc_y_in[t][:].rearrange("(h p) c -> p h c", p=128),
                                yTt[:])
                            nc.gpsimd.collective_compute(
                                "AllGather", OP.bypass,
                                replica_groups=[list(range(CORES))],
                                ins=[cc_y_in[t][:].opt()],
                                outs=[cc_y_out[t][:].opt()])

                # ===== Stages C+D merged: per t-chunk pipeline:
                # o-proj -> x1 -> norm AR -> xn1 AG -> fc -> proj =====
                with tc_.tile_pool(name="sCD", bufs=1) as sCD, \
                     tc_.tile_pool(name="sCD2", bufs=2) as sCD2, \
                     tc_.tile_pool(name="psO", bufs=2, space="PSUM") as psO, \
                     tc_.tile_pool(name="psR3", bufs=2, space="PSUM") as psR3, \
                     tc_.tile_pool(name="psH", bufs=2, space="PSUM") as psH, \
                     tc_.tile_pool(name="psM", bufs=2, space="PSUM") as psM:
                    nsum = sCD.tile([1, T], f32)
                    mlpr = mlp_d.ap().rearrange("(n p) t -> p n t", p=128)
                    xsr = xTs_d.ap().rearrange("(n p) t -> p n t", p=128)
                    for t in range(NT):
                        ts = slice(t * CH, (t + 1) * CH)
                        ygt = sCD2.tile([128, ND, CH], bf, tag="ygt", bufs=1)
                        nc.sync.dma_start(
                            ygt[:],
                            cc_y_out[t][:].rearrange("(n p) c -> p n c", p=128))
                        xst = sCD2.tile([128, NE, CH], f32, tag="xst")
                        nc.sync.dma_start(xst[:], xsr[:, :, ts])
                        for m in range(NE):
                            po = psO.tile([128, CH], f32, tag="o")
                            for n in range(ND):
                                nc.tensor.matmul(
                                    po[:], wo[:, n, m * 128:(m + 1) * 128],
                                    ygt[:, n], start=(n == 0), stop=(n == ND - 1))
                            nc.vector.tensor_tensor(x1T[:, m, ts], po[:],
                                                    xst[:, m], OP.add)
                        pr3 = psR3.tile([1, CH], f32, tag="r3")
                        for m in range(NE):
                            sq3 = sCD2.tile([128, CH], f32r, tag="sq3")
                            nc.scalar.activation(sq3[:], x1T[:, m, ts], AF.Square)
                            nc.tensor.matmul(pr3[:], ones[:], sq3[:],
                                             start=(m == 0), stop=(m == NE - 1))
                        nc.vector.tensor_copy(nsum[:, ts], pr3[:])
                        nc.sync.dma_start(cc_n_in[t][:], nsum[:, ts])
                        nc.gpsimd.collective_compute(
                            "AllReduce", OP.add,
                            replica_groups=[list(range(CORES))],
                            ins=[cc_n_in[t][:].opt()], outs=[cc_n_out[t][:].opt()])
                        nfull = sCD2.tile([1, CH], f32, tag="nfull")
                        nc.sync.dma_start(nfull[:], cc_n_out[t][:])
                        sd3 = sCD2.tile([1, CH], f32, tag="sd3")
                        nc.scalar.activation(sd3[:], nfull[:], AF.Sqrt,
                                             bias=epsb[0:1, :], scale=1.0 / D)
                        rcp3 = sCD2.tile([1, CH], f32, tag="rcp3")
                        nc.vector.reciprocal_approx_fast(rcp3[:], sd3[:])
                        rep3 = sCD2.tile([128, CH], f32, tag="rep3")
                        nc.gpsimd.partition_broadcast(rep3[:], rcp3[:])
                        xn1 = sCD2.tile([128, NE, CH], bf, tag="xn1")
                        for m in range(NE):
                            nc.vector.tensor_tensor(xn1[:, m], x1T[:, m, ts],
                                                    rep3[:], OP.mult)
                        nc.sync.dma_start(
                            cc_x_in[t][:].rearrange("(n p) c -> p n c", p=128),
                            xn1[:])
                        nc.gpsimd.collective_compute(
                            "AllGather", OP.bypass,
                            replica_groups=[list(range(CORES))],
                            ins=[cc_x_in[t][:].opt()], outs=[cc_x_out[t][:].opt()])
                        xnt = sCD2.tile([128, ND, CH], bf, tag="xnt", bufs=1)
                        nc.sync.dma_start(
                            xnt[:],
                            cc_x_out[t][:].rearrange("(n p) c -> p n c", p=128))
                        hT = sCD2.tile([128, NF, CH], bf, tag="hT", bufs=1)
                        for mf in range(NF):
                            ph = psH.tile([128, CH], f32, tag="h")
                            for n in range(ND):
                                nc.tensor.matmul(
                                    ph[:], wfc[:, n, mf * 128:(mf + 1) * 128],
                                    xnt[:, n], start=(n == 0), stop=(n == ND - 1))
                            hr = sCD2.tile([128, CH], f32, tag="hr")
                            nc.scalar.activation(hr[:], ph[:], AF.Relu)
                            nc.vector.tensor_tensor(hT[:, mf], hr[:], hr[:],
                                                    OP.mult)
                        for md in range(ND):
                            pm = psM.tile([128, CH], f32, tag="m")
                            for mf in range(NF):
                                nc.tensor.matmul(
                                    pm[:], wpr[:, mf, md * 128:(md + 1) * 128],
                                    hT[:, mf], start=(mf == 0), stop=(mf == NF - 1))
                            ot = sCD2.tile([128, CH], f32, tag="ot")
                            nc.vector.tensor_copy(ot[:], pm[:])
                            nc.sync.dma_start(mlpr[:, md, ts], ot[:])
                    nc.sync.dma_start(
                        x1s_d.ap().rearrange("(n p) t -> p n t", p=128), x1T[:])

    nc.compile()
    return nc


def kernel(x, ve, qkvo_w, attn_gate_w, ve_gate_w, c_fc, c_proj,
           sa_lambdas, cos, sin, attn_scale, docs, key_offset):
    global LAST_RESULT
    x = np.asarray(x, np.float32)
    ve = np.asarray(ve, np.float32)
    qkvo_w = np.asarray(qkvo_w, np.float32)
    attn_gate_w = np.asarray(attn_gate_w, np.float32)
    ve_gate_w = np.asarray(ve_gate_w, np.float32)
    c_fc = np.asarray(c_fc, np.float32)
    c_proj = np.asarray(c_proj, np.float32)
    sa = np.asarray(sa_lambdas, np.float32)
    docs = np.asarray(docs)
    ko = int(np.asarray(key_offset))
    scale = float(np.asarray(attn_scale).reshape(-1)[0])

    band = _attn_band(docs)
    key = (docs.tobytes(), scale, ko)
    if key not in _CACHE:
        _CACHE[key] = _build(band, scale, ko)
    nc = _CACHE[key]

    xT = np.ascontiguousarray(x[0].T)                       # [D, T] f32
    w_qkv = sa[0] * qkvo_w[:3 * D]                          # [3D, D]
    w_o = sa[1] * qkvo_w[3 * D:]                            # [D, D]
    cosT = np.asarray(cos, np.float32).T
    sinT = np.asarray(sin, np.float32).T
    ropeA = np.ascontiguousarray(np.concatenate([cosT, sinT], 0)).astype(BF16)
    ropeB = np.ascontiguousarray(np.concatenate([sinT, cosT], 0)).astype(BF16)
    identb = np.eye(128).astype(BF16)
    ones = np.ones((128, 1), np.float32)
    onesb = np.ones((128, 1)).astype(BF16)
    epsb_np = np.full((128, 1), EPS, np.float32)
    xT_bf = xT.astype(BF16)

    mask_list = []
    for tcc, tiles in enumerate(band):
        for (st, full, m) in tiles:
            if not full:
                mask_list.append(m)
    if not mask_list:
        mask_list = [np.zeros((128, CH), np.float32)]
    masks = np.ascontiguousarray(np.stack(mask_list, 0)).astype(BF16)

    in_maps = []
    for c in range(CORES):
        g0 = c * HPC
        rows = []
        for blk in range(3):                                # q, k, v blocks
            for h in range(HPC):
                g = g0 + h
                rows.append(w_qkv[blk * D + g * HD:blk * D + (g + 1) * HD])
        wqkvT = np.ascontiguousarray(np.concatenate(rows, 0).T).astype(BF16)
        woT = np.ascontiguousarray(w_o[c * DSH:(c + 1) * DSH].T).astype(BF16)
        wfcT = np.ascontiguousarray(c_fc[c * FSH:(c + 1) * FSH].T).astype(BF16)
        wpr = np.ascontiguousarray(c_proj[c * FSH:(c + 1) * FSH]).astype(BF16)
        wgT = np.zeros((D, 128), np.float32)
        wgT[:, 0] = ve_gate_w[g0]
        wgT[:, 32] = ve_gate_w[g0 + 1]
        wgT[:, 64] = attn_gate_w[g0]
        wgT[:, 96] = attn_gate_w[g0 + 1]
        wgT = wgT.astype(BF16)
        veS = np.ascontiguousarray(
            ve[0][:, g0 * HD:(g0 + HPC) * HD]).astype(BF16)         # [T, E]
        xTs = np.ascontiguousarray(xT[c * DSH:(c + 1) * DSH])       # [DSH, T]
        in_maps.append(dict(
            xT=xT_bf, xTs=xTs, wqkvT=wqkvT, woT=woT, wfcT=wfcT, wpr=wpr,
            wgT=wgT, veS=veS, ropeA=ropeA, ropeB=ropeB, identb=identb,
            ones=ones, onesb=onesb, epsb=epsb_np, masks=masks))

    res = run_bass_kernel_spmd(nc, in_maps, core_ids=list(range(CORES)))
    LAST_RESULT = res

    outT = res.results[0]["mlp_partialT"].astype(np.float64)
    for c in range(1, CORES):
        outT += res.results[c]["mlp_partialT"]
    for c in range(CORES):
        outT[c * DSH:(c + 1) * DSH] += res.results[c]["x1T_shard"]
    return np.ascontiguousarray(outT.T).astype(np.float32).reshape(1, T, D)
